# revision 50
# baseline (speedup 1.0000x reference)
"""SAGAN-style attention block on 8 Trainium2 NeuronCores.

Math (per batch b):
  theta = W_theta @ x + b_theta            [8, 4096]
  phi   = maxpool2(W_phi @ x + b_phi)      [8, 1024]
  g     = maxpool2(W_g   @ x + b_g)        [32, 1024]
  E[m,n] = exp(S^T[m,n]), S^T[m,n] = sum_c phi[c,m] theta[c,n]
  O_aug = [g; ones] @ E                    [33, 4096]  (row 32 = softmax denom)
  o     = O_aug[0:32] / O_aug[32]
  out   = x + gamma*(W_o @ o + b_o)

Sharding: batch dim (16) split across 8 cores, 2 batches/core; weights
replicated.  No max-subtraction in softmax: |S| <= ~3 so exp is safe.

Schedule: the exp instructions on the Activation engine are the roofline
(64 x [128,1024] per core); everything else is arranged so Act never
stalls.  One flat software pipeline runs over all 64 (batch, chunk,
group) slots: slot K emits S^T(K) -> exp(K) -> O(K-2) with the next
batch's projection matmuls, the PE transposes, and the per-chunk
normalize/output tails interleaved into the PE/DVE/Pool slack.  Maxpools
run on GpSimd; per-chunk tails stagger recip/bcast/mult two slots ahead
of the W_o matmul so the in-order PE queue never head-of-line blocks.
Residual and output are bf16 (abs err ~0.02 vs the 0.1 tolerance).
"""

import contextlib

import ml_dtypes
import numpy as np

import concourse.bass as bass
import concourse.mybir as mybir
import concourse.tile as tile
from concourse import bacc
from concourse.bass_utils import run_bass_kernel_spmd
from concourse.masks import make_identity

B, C, H, W = 16, 64, 64, 64
N = H * W            # 4096 pixels
M = N // 4           # 1024 pooled pixels
NCORES = 8
BPC = B // NCORES    # 2 batches per core
CT = C // 8          # 8 theta/phi channels
CG = C // 2          # 32 g channels
NC = 512             # n-chunk width
NCH = N // NC        # 8 chunks per batch
MT = 128             # m-tile (partitions)
MTS = M // MT        # 8 m-tiles
GRP = 2              # m-tiles per exp group ([128, 1024] PSUM staging)
NG = MTS // GRP      # 4 groups per chunk
SLOTS = BPC * NCH * NG  # 32 groups/batch, 64 per core

F32 = mybir.dt.float32
BF16 = mybir.dt.bfloat16
FP8 = mybir.dt.float8e4
EXP = mybir.ActivationFunctionType.Exp
MAX = mybir.AluOpType.max
ADD = mybir.AluOpType.add
MULT = mybir.AluOpType.mult
DBLROW = mybir.MatmulPerfMode.DoubleRow
USE_FP8_O = True   # fp8 DoubleRow O-matmul path vs bf16 pair path
GAW = 64           # g_aug^T padded m-tile width (DoubleRow ldweights cols)


def build_bass(loop_n=None, repeat=1):
    """loop_n: if set, wrap the whole computation in a hardware loop that
    repeats it loop_n times; repeat duplicates the body inside one
    iteration (benchmarking only)."""
    nc = bacc.Bacc("TRN2", target_bir_lowering=False, debug=False)

    xbf_d = nc.dram_tensor("xbf", [BPC, C, N], BF16, kind="ExternalInput").ap()
    # all bf16 constants in one DMA: cols 0:104 W_all^T (64 rows),
    # 104:168 gamma*W_o^T (32 rows)
    cbf_d = nc.dram_tensor("const_bf", [104, 168], BF16,
                           kind="ExternalInput").ap()
    # f32 constants: col 0 = biases (104 rows), col 1 = gamma*b_o (64 rows)
    cf32_d = nc.dram_tensor("const_f32", [104, 2], F32,
                            kind="ExternalInput").ap()
    out_d = nc.dram_tensor("out", [BPC, C, N], BF16, kind="ExternalOutput").ap()

    with tile.TileContext(nc) as tc:
        with (
            tc.tile_pool(name="consts", bufs=1) as consts,
            tc.tile_pool(name="perbatch", bufs=2) as pb,
            tc.tile_pool(name="epool", bufs=4) as ep,
            tc.tile_pool(name="small", bufs=2) as sm,
            tc.tile_pool(name="stpsum", bufs=2, space="PSUM") as stp,
            tc.tile_pool(name="otpsum", bufs=2, space="PSUM") as otp,
            tc.tile_pool(name="pjutpsum", bufs=2, space="PSUM") as pjut,
        ):
            cbf = consts.tile([104, 168], BF16)
            nc.sync.dma_start(out=cbf, in_=cbf_d)
            # f32 constants go through the software DGE (Pool) to keep the
            # hardware DGE queue clear for the first xbf chunk
            cf32 = consts.tile([104, 2], F32)
            nc.gpsimd.dma_start(out=cf32, in_=cf32_d)
            wallt = cbf[0:C, 0:104]
            wot = cbf[0:CG, 104:168]
            ball = cf32[:, 0:1]
            gbo = cf32[0:C, 1:2]
            # identity built on Pool at t=0 (no DMA dependency) so the PE
            # warm-up chain can start immediately
            ident = consts.tile([CG, CG], BF16)
            make_identity(nc, ident)

            # persistent per-batch g_aug^T tiles: the ones column is
            # written once, outside the loop ([128, 8] strided - cheap)
            gaTs = []
            for b in range(BPC):
                if USE_FP8_O:
                    # 64-wide m-tile stride keeps the fp8 DoubleRow
                    # ldweights pattern ISA-legal at half the weight-load
                    # cost of 128; pad columns stay zero
                    gaT = consts.tile([MT, MTS, GAW], FP8, name=f"gaT{b}")
                    nc.vector.memset(gaT, 0.0)
                else:
                    gaT = consts.tile([MT, MTS, 33], BF16, name=f"gaT{b}")
                nc.vector.memset(gaT[:, :, 32], 1.0)
                gaTs.append(gaT)

            env = dict(
                xbf_d=xbf_d, out_d=out_d, wallt=wallt, ball=ball, wot=wot,
                gbo=gbo, ident=ident, gaTs=gaTs,
                pb=pb, ep=ep, sm=sm, stp=stp, otp=otp, pjut=pjut,
            )
            loop_cm = (tc.For_i(0, loop_n, 1) if loop_n
                       else contextlib.nullcontext())
            with loop_cm:
                for _ in range(repeat):
                    core_body(nc, env)
    nc.compile()
    return nc


def core_body(nc, env):
    xbf_d, out_d = env["xbf_d"], env["out_d"]
    wallt, ball, wot, gbo, ident = (env["wallt"], env["ball"], env["wot"],
                                    env["gbo"], env["ident"])
    gaTs = env["gaTs"]
    pb, ep, sm = env["pb"], env["ep"], env["sm"]
    stp, otp, pjut = env["stp"], env["otp"], env["pjut"]

    xbfs = [pb.tile([C, N], BF16, tag="xbf", name=f"xbf{b}")
            for b in range(BPC)]
    projs = [pb.tile([104, N], BF16, tag="proj", name=f"proj{b}")
             for b in range(BPC)]
    pgps = [pb.tile([40, M], BF16, tag="pgp", name=f"pgp{b}")
            for b in range(BPC)]
    onorms = [pb.tile([CG, N], BF16, tag="onorm", name=f"onorm{b}")
              for b in range(BPC)]
    outbs = [pb.tile([C, N], BF16, tag="outb", name=f"outb{b}")
             for b in range(BPC)]

    # first transfer small so projection chunk 0 starts ASAP
    nc.sync.dma_start(out=xbfs[0][:, 0:NC], in_=xbf_d[0][:, 0:NC])
    nc.sync.dma_start(out=xbfs[0][:, NC:N // 2], in_=xbf_d[0][:, NC:N // 2])
    nc.sync.dma_start(out=xbfs[0][:, N // 2:N], in_=xbf_d[0][:, N // 2:N])
    nc.sync.dma_start(out=xbfs[1][:, 0:N // 2], in_=xbf_d[1][:, 0:N // 2])
    nc.sync.dma_start(out=xbfs[1][:, N // 2:N], in_=xbf_d[1][:, N // 2:N])

    # PE p-state warm-up: ~3us of tiny transposes so the projection
    # matmuls run at full clock as soon as their data lands
    wup = pjut.tile([CG, CG], BF16, tag="pjut", name="wup")
    for _ in range(75):
        nc.tensor.transpose(wup, ident, ident)

    def proj_step(b, c):
        # theta/phi/g chunk c: W_all @ x + bias, then 2x2 maxpool of the
        # g/phi rows (proj rows 64:104 -> pgp rows 0:40) on GpSimd
        cs = slice(c * NC, (c + 1) * NC)
        pj = pjut.tile([104, NC], F32, tag="pjut", name=f"pj{b}_{c}")
        nc.tensor.matmul(pj, wallt, xbfs[b][:, cs], start=True, stop=True)
        nc.vector.tensor_scalar_add(out=projs[b][:, cs], in0=pj,
                                    scalar1=ball)
        mjs = slice(c * 128, (c + 1) * 128)
        ch = projs[b][64:104, cs].rearrange("p (w t) -> p w t", t=2)
        wm = sm.tile([40, 256], BF16, tag="wm", name=f"wm{b}_{c}")
        nc.vector.tensor_tensor(out=wm, in0=ch[:, :, 0], in1=ch[:, :, 1],
                                op=MAX)
        wmv = wm.rearrange("p (h t w) -> p h t w", t=2, w=W // 2)
        po = pgps[b][:, mjs].rearrange("p (h w) -> p h w", w=W // 2)
        nc.vector.tensor_tensor(out=po, in0=wmv[:, :, 0, :],
                                in1=wmv[:, :, 1, :], op=MAX)

    def tr_step(b, c):
        # transpose pooled-g m-tile c into the g_aug^T layout (fp8 cast
        # happens in the copy; the O matmuls run fp8 DoubleRow)
        gt = pjut.tile([MT, CG], BF16, tag="pjut", name=f"gt{b}_{c}")
        nc.tensor.transpose(gt, pgps[b][0:CG, c * MT:(c + 1) * MT], ident)
        nc.vector.tensor_copy(out=gaTs[b][:, c, 0:32], in_=gt)

    ets = {}   # global slot -> et tile
    ots = {}   # (b, j) -> O_aug accumulator

    def st_exp(K):
        b, k = K // (NCH * NG), K % (NCH * NG)
        j, g = k // NG, k % NG
        js = slice(j * NC, (j + 1) * NC)
        st = stp.tile([MT, GRP * NC], F32, tag="st", name=f"st{K}")
        for t in range(GRP):
            i = GRP * g + t
            nc.tensor.matmul(
                st[:, t * NC:(t + 1) * NC],
                pgps[b][32:40, i * MT:(i + 1) * MT],
                projs[b][32:40, js],
                start=True, stop=True)
        et = ep.tile([MT, GRP * NC], FP8 if USE_FP8_O else BF16,
                     tag="et", name=f"et{K}")
        nc.scalar.activation(out=et, in_=st, func=EXP)
        ets[K] = et

    def o_step(K):
        # one fp8 DoubleRow matmul contracts both m-tiles of the group:
        # lhsT [128, 2, 33], rhs [128, 2, 512] -> out [33, 512]
        b, k = K // (NCH * NG), K % (NCH * NG)
        j, g = k // NG, k % NG
        if g == 0:
            ots[(b, j)] = otp.tile([GAW if USE_FP8_O else 33, NC], F32,
                                   tag="ot", name=f"ot{b}_{j}")
        ot = ots[(b, j)]
        et = ets.pop(K)
        if USE_FP8_O:
            nc.tensor.matmul(ot, gaTs[b][:, GRP * g:GRP * (g + 1), :],
                             et.rearrange("p (two n) -> p two n", two=GRP),
                             start=(g == 0), stop=(g == NG - 1),
                             perf_mode=DBLROW)
        else:
            for t in range(GRP):
                i = GRP * g + t
                nc.tensor.matmul(ot, gaTs[b][:, i, :],
                                 et[:, t * NC:(t + 1) * NC],
                                 start=(i == 0), stop=(i == MTS - 1))

    def tail_norm(b, j):
        # o_norm = O[0:32] * (1/denom): recip (DVE) -> partition broadcast
        # (Pool) -> multiply (DVE)
        js = slice(j * NC, (j + 1) * NC)
        ot = ots[(b, j)]
        rs = sm.tile([1, NC], F32, tag="rs", name=f"rs{b}_{j}")
        nc.vector.reciprocal(out=rs, in_=ot[32:33, :])
        r32 = sm.tile([CG, NC], F32, tag="r32", name=f"r32{b}_{j}")
        nc.gpsimd.partition_broadcast(r32, rs)
        nc.vector.tensor_tensor(out=onorms[b][:, js], in0=ot[0:32, :],
                                in1=r32, op=MULT)

    def tail_out(b, j):
        # out = (gamma*W_o @ o + gamma*b_o) + x  (gamma folded into wot,
        # gamma*b_o applied as a per-partition scalar in the final add)
        js = slice(j * NC, (j + 1) * NC)
        del ots[(b, j)]
        ut = pjut.tile([C, NC], F32, tag="pjut", name=f"ut{b}_{j}")
        nc.tensor.matmul(ut, wot, onorms[b][:, js], start=True, stop=True)
        nc.vector.scalar_tensor_tensor(out=outbs[b][:, js], in0=ut,
                                       scalar=gbo, in1=xbfs[b][:, js],
                                       op0=ADD, op1=ADD)
        if j == NCH // 2 - 1:
            nc.sync.dma_start(out=out_d[b][:, 0:N // 2],
                              in_=outbs[b][:, 0:N // 2])
        elif b == BPC - 1 and j == NCH - 2:
            # keep the very last transfer small: it sits on the drain path
            nc.sync.dma_start(out=out_d[b][:, N // 2:N - NC],
                              in_=outbs[b][:, N // 2:N - NC])
        elif j == NCH - 1:
            if b == BPC - 1:
                nc.sync.dma_start(out=out_d[b][:, N - NC:N],
                                  in_=outbs[b][:, N - NC:N])
            else:
                nc.sync.dma_start(out=out_d[b][:, N // 2:N],
                                  in_=outbs[b][:, N // 2:N])

    # ---- emission schedule ----
    KPB = NCH * NG  # 32 slots per batch
    # batch 1 projection steps paced through batch 0's attention slots:
    # even slots only, so the PE extras never collide with the W_o
    # matmuls of the per-chunk tails (odd slots)
    b1_proj = {4 + 3 * c: c for c in range(NCH)}
    b1_tr = {6 + 3 * c: c for c in range(NCH)}

    proj_step(0, 0)
    proj_step(0, 1)
    for K in range(SLOTS):
        if K <= 3 and K >= 1:
            # batch-0 fill: remaining projection chunks ahead of their
            # first use; transposes one slot after their maxpools
            proj_step(0, 2 * K)
            proj_step(0, 2 * K + 1)
        st_exp(K)
        if 1 <= K <= 4:
            tr_step(0, 2 * K - 2)
            tr_step(0, 2 * K - 1)
        if K in b1_proj:
            proj_step(1, b1_proj[K])
        if K in b1_tr:
            tr_step(1, b1_tr[K])
        if K - 2 >= 0:
            o_step(K - 2)
        # per-chunk tails: normalize 2 slots after the chunk's last O
        # matmul was emitted, W_o matmul + residual 2 slots later again
        for b in range(BPC):
            k = K - b * KPB
            if k >= 5 and (k - 5) % NG == 0 and (k - 5) // NG < NCH:
                tail_norm(b, (k - 5) // NG)
            if k >= 7 and (k - 7) % NG == 0 and (k - 7) // NG < NCH:
                tail_out(b, (k - 7) // NG)
    # epilogue: flush the last two O groups, then drain the final chunk's
    # tail as two 256-col half-chains so DVE/Pool/PE/DMA pipeline
    o_step(SLOTS - 2)
    o_step(SLOTS - 1)
    b, j = BPC - 1, NCH - 1
    ot = ots[(b, j)]
    HC = NC // 2
    rss, r32s = [], []
    for h in range(2):
        hs = slice(h * HC, (h + 1) * HC)
        rs = sm.tile([1, HC], F32, tag="rs", name=f"rse{h}")
        nc.vector.reciprocal(out=rs, in_=ot[32:33, hs])
        rss.append(rs)
    for h in range(2):
        r32 = sm.tile([CG, HC], F32, tag="r32", name=f"r32e{h}")
        nc.gpsimd.partition_broadcast(r32, rss[h])
        r32s.append(r32)
    for h in range(2):
        lo = j * NC + h * HC
        hs = slice(h * HC, (h + 1) * HC)
        gs = slice(lo, lo + HC)
        nc.vector.tensor_tensor(out=onorms[b][:, gs], in0=ot[0:32, hs],
                                in1=r32s[h], op=MULT)
        ut = pjut.tile([C, HC], F32, tag="pjut", name=f"ute{h}")
        nc.tensor.matmul(ut, wot, onorms[b][:, gs], start=True, stop=True)
        nc.vector.scalar_tensor_tensor(out=outbs[b][:, gs], in0=ut,
                                       scalar=gbo, in1=xbfs[b][:, gs],
                                       op0=ADD, op1=ADD)
        nc.sync.dma_start(out=out_d[b][:, gs], in_=outbs[b][:, gs])
    del ots[(b, j)]


_NC_CACHE = None


def _get_nc():
    global _NC_CACHE
    if _NC_CACHE is None:
        _NC_CACHE = build_bass()
    return _NC_CACHE


def prep_in_maps(inputs, W_theta, b_theta, W_phi, b_phi, W_g, b_g, W_o, b_o,
                 gamma, **_unused):
    inputs = np.asarray(inputs, np.float32)
    W_all = np.zeros((104, C), np.float32)
    W_all[32:32 + CT] = np.asarray(W_theta, np.float32)
    W_all[64:64 + CG] = np.asarray(W_g, np.float32)
    W_all[96:96 + CT] = np.asarray(W_phi, np.float32)
    g = np.float32(np.asarray(gamma, np.float32))

    const_bf = np.zeros((104, 168), np.float32)
    const_bf[0:C, 0:104] = W_all.T
    const_bf[0:CG, 104:168] = np.asarray(W_o, np.float32).T * g
    const_bf = np.ascontiguousarray(const_bf.astype(ml_dtypes.bfloat16))

    const_f32 = np.zeros((104, 2), np.float32)
    const_f32[32:32 + CT, 0] = np.asarray(b_theta, np.float32)
    const_f32[64:64 + CG, 0] = np.asarray(b_g, np.float32)
    const_f32[96:96 + CT, 0] = np.asarray(b_phi, np.float32)
    const_f32[0:C, 1] = np.asarray(b_o, np.float32) * g
    const_f32 = np.ascontiguousarray(const_f32)

    xbf = inputs.reshape(B, C, N).astype(ml_dtypes.bfloat16)
    in_maps = []
    for c in range(NCORES):
        in_maps.append({
            "xbf": np.ascontiguousarray(xbf[c * BPC:(c + 1) * BPC]),
            "const_bf": const_bf,
            "const_f32": const_f32,
        })
    return in_maps


def kernel(**inputs):
    in_maps = prep_in_maps(**inputs)
    nc = _get_nc()
    res = run_bass_kernel_spmd(nc, in_maps, core_ids=list(range(NCORES)))
    out = np.concatenate([res.results[c]["out"] for c in range(NCORES)],
                         axis=0)
    return out.reshape(B, C, H, W).astype(np.float32)


if __name__ == "__main__":
    rng = np.random.default_rng(0)
    ins = {
        "inputs": rng.standard_normal((B, C, H, W)).astype(np.float32),
        "W_theta": (rng.standard_normal((CT, C)) * 0.05).astype(np.float32),
        "b_theta": np.zeros(CT, np.float32),
        "W_phi": (rng.standard_normal((CT, C)) * 0.05).astype(np.float32),
        "b_phi": np.zeros(CT, np.float32),
        "W_g": (rng.standard_normal((CG, C)) * 0.05).astype(np.float32),
        "b_g": np.zeros(CG, np.float32),
        "W_o": (rng.standard_normal((C, CG)) * 0.05).astype(np.float32),
        "b_o": np.zeros(C, np.float32),
        "gamma": np.float32(0.5),
    }
    print(kernel(**ins).shape)


# revision 52
# speedup vs baseline: 1.0183x; 1.0183x over previous
"""SAGAN-style attention block on 8 Trainium2 NeuronCores.

Math (per batch b):
  theta = W_theta @ x + b_theta            [8, 4096]
  phi   = maxpool2(W_phi @ x + b_phi)      [8, 1024]
  g     = maxpool2(W_g   @ x + b_g)        [32, 1024]
  E[m,n] = exp(S^T[m,n]), S^T[m,n] = sum_c phi[c,m] theta[c,n]
  O_aug = [g; ones] @ E                    [33, 4096]  (row 32 = softmax denom)
  o     = O_aug[0:32] / O_aug[32]
  out   = x + gamma*(W_o @ o + b_o)

Sharding: batch dim (16) split across 8 cores, 2 batches/core; weights
replicated.  No max-subtraction in softmax: |S| <= ~3 so exp is safe.

Schedule: the exp instructions on the Activation engine are the roofline
(64 x [128,1024] per core); everything else is arranged so Act never
stalls.  One flat software pipeline runs over all 64 (batch, chunk,
group) slots: slot K emits S^T(K) -> exp(K) -> O(K-2) with the next
batch's projection matmuls, the PE transposes, and the per-chunk
normalize/output tails interleaved into the PE/DVE/Pool slack.  Maxpools
run on GpSimd; per-chunk tails stagger recip/bcast/mult two slots ahead
of the W_o matmul so the in-order PE queue never head-of-line blocks.
Residual and output are bf16 (abs err ~0.02 vs the 0.1 tolerance).
"""

import contextlib

import ml_dtypes
import numpy as np

import concourse.bass as bass
import concourse.mybir as mybir
import concourse.tile as tile
from concourse import bacc
from concourse.bass_utils import run_bass_kernel_spmd
from concourse.masks import make_identity

B, C, H, W = 16, 64, 64, 64
N = H * W            # 4096 pixels
M = N // 4           # 1024 pooled pixels
NCORES = 8
BPC = B // NCORES    # 2 batches per core
CT = C // 8          # 8 theta/phi channels
CG = C // 2          # 32 g channels
NC = 512             # n-chunk width
NCH = N // NC        # 8 chunks per batch
MT = 128             # m-tile (partitions)
MTS = M // MT        # 8 m-tiles
GRP = 2              # m-tiles per exp group ([128, 1024] PSUM staging)
NG = MTS // GRP      # 4 groups per chunk
SLOTS = BPC * NCH * NG  # 32 groups/batch, 64 per core

F32 = mybir.dt.float32
BF16 = mybir.dt.bfloat16
FP8 = mybir.dt.float8e4
EXP = mybir.ActivationFunctionType.Exp
MAX = mybir.AluOpType.max
ADD = mybir.AluOpType.add
MULT = mybir.AluOpType.mult
DBLROW = mybir.MatmulPerfMode.DoubleRow
USE_FP8_O = True   # fp8 DoubleRow O-matmul path vs bf16 pair path
GAW = 64           # g_aug^T padded m-tile width (DoubleRow ldweights cols)


def build_bass(loop_n=None, repeat=1):
    """loop_n: if set, wrap the whole computation in a hardware loop that
    repeats it loop_n times; repeat duplicates the body inside one
    iteration (benchmarking only)."""
    nc = bacc.Bacc("TRN2", target_bir_lowering=False, debug=False)

    xbf_d = nc.dram_tensor("xbf", [BPC, C, N], BF16, kind="ExternalInput").ap()
    # all bf16 constants in one DMA: cols 0:104 W_all^T (64 rows),
    # 104:168 gamma*W_o^T (32 rows)
    cbf_d = nc.dram_tensor("const_bf", [104, 168], BF16,
                           kind="ExternalInput").ap()
    # f32 constants: col 0 = biases (104 rows), col 1 = gamma*b_o (64 rows)
    cf32_d = nc.dram_tensor("const_f32", [104, 2], F32,
                            kind="ExternalInput").ap()
    out_d = nc.dram_tensor("out", [BPC, C, N], BF16, kind="ExternalOutput").ap()

    with tile.TileContext(nc) as tc:
        with (
            tc.tile_pool(name="consts", bufs=1) as consts,
            tc.tile_pool(name="perbatch", bufs=2) as pb,
            tc.tile_pool(name="epool", bufs=4) as ep,
            tc.tile_pool(name="small", bufs=2) as sm,
            tc.tile_pool(name="stpsum", bufs=2, space="PSUM") as stp,
            tc.tile_pool(name="otpsum", bufs=2, space="PSUM") as otp,
            tc.tile_pool(name="pjutpsum", bufs=2, space="PSUM") as pjut,
        ):
            cbf = consts.tile([104, 168], BF16)
            nc.sync.dma_start(out=cbf, in_=cbf_d)
            # f32 constants go through the software DGE (Pool) to keep the
            # hardware DGE queue clear for the first xbf chunk
            cf32 = consts.tile([104, 2], F32)
            nc.gpsimd.dma_start(out=cf32, in_=cf32_d)
            wallt = cbf[0:C, 0:104]
            wot = cbf[0:CG, 104:168]
            ball = cf32[:, 0:1]
            gbo = cf32[0:C, 1:2]
            # identity built on Pool at t=0 (no DMA dependency) so the PE
            # warm-up chain can start immediately
            ident = consts.tile([CG, CG], BF16)
            make_identity(nc, ident)

            # persistent per-batch g_aug^T tiles: the ones column is
            # written once, outside the loop ([128, 8] strided - cheap)
            gaTs = []
            for b in range(BPC):
                if USE_FP8_O:
                    # 64-wide m-tile stride keeps the fp8 DoubleRow
                    # ldweights pattern ISA-legal at half the weight-load
                    # cost of 128; pad columns stay zero
                    gaT = consts.tile([MT, MTS, GAW], FP8, name=f"gaT{b}")
                    nc.vector.memset(gaT, 0.0)
                else:
                    gaT = consts.tile([MT, MTS, 33], BF16, name=f"gaT{b}")
                nc.vector.memset(gaT[:, :, 32], 1.0)
                gaTs.append(gaT)

            env = dict(
                xbf_d=xbf_d, out_d=out_d, wallt=wallt, ball=ball, wot=wot,
                gbo=gbo, ident=ident, gaTs=gaTs,
                pb=pb, ep=ep, sm=sm, stp=stp, otp=otp, pjut=pjut,
            )
            loop_cm = (tc.For_i(0, loop_n, 1) if loop_n
                       else contextlib.nullcontext())
            with loop_cm:
                for _ in range(repeat):
                    core_body(nc, env)
    nc.compile()
    return nc


def core_body(nc, env):
    xbf_d, out_d = env["xbf_d"], env["out_d"]
    wallt, ball, wot, gbo, ident = (env["wallt"], env["ball"], env["wot"],
                                    env["gbo"], env["ident"])
    gaTs = env["gaTs"]
    pb, ep, sm = env["pb"], env["ep"], env["sm"]
    stp, otp, pjut = env["stp"], env["otp"], env["pjut"]

    xbfs = [pb.tile([C, N], BF16, tag="xbf", name=f"xbf{b}")
            for b in range(BPC)]
    projs = [pb.tile([104, N], BF16, tag="proj", name=f"proj{b}")
             for b in range(BPC)]
    pgps = [pb.tile([40, M], BF16, tag="pgp", name=f"pgp{b}")
            for b in range(BPC)]
    onorms = [pb.tile([CG, N], BF16, tag="onorm", name=f"onorm{b}")
              for b in range(BPC)]
    outbs = [pb.tile([C, N], BF16, tag="outb", name=f"outb{b}")
             for b in range(BPC)]

    # first transfer small so projection chunk 0 starts ASAP
    nc.sync.dma_start(out=xbfs[0][:, 0:NC], in_=xbf_d[0][:, 0:NC])
    nc.sync.dma_start(out=xbfs[0][:, NC:N // 2], in_=xbf_d[0][:, NC:N // 2])
    nc.sync.dma_start(out=xbfs[0][:, N // 2:N], in_=xbf_d[0][:, N // 2:N])
    nc.sync.dma_start(out=xbfs[1][:, 0:N // 2], in_=xbf_d[1][:, 0:N // 2])
    nc.sync.dma_start(out=xbfs[1][:, N // 2:N], in_=xbf_d[1][:, N // 2:N])

    # PE p-state warm-up: ~3us of tiny transposes so the projection
    # matmuls run at full clock as soon as their data lands
    wup = pjut.tile([CG, CG], BF16, tag="pjut", name="wup")
    for _ in range(75):
        nc.tensor.transpose(wup, ident, ident)

    def proj_step(b, c):
        # theta/phi/g chunk c: W_all @ x + bias, then 2x2 maxpool of the
        # g/phi rows (proj rows 64:104 -> pgp rows 0:40) on GpSimd
        cs = slice(c * NC, (c + 1) * NC)
        pj = pjut.tile([104, NC], F32, tag="pjut", name=f"pj{b}_{c}")
        nc.tensor.matmul(pj, wallt, xbfs[b][:, cs], start=True, stop=True)
        nc.vector.tensor_scalar_add(out=projs[b][:, cs], in0=pj,
                                    scalar1=ball)
        mjs = slice(c * 128, (c + 1) * 128)
        ch = projs[b][64:104, cs].rearrange("p (w t) -> p w t", t=2)
        wm = sm.tile([40, 256], BF16, tag="wm", name=f"wm{b}_{c}")
        nc.vector.tensor_tensor(out=wm, in0=ch[:, :, 0], in1=ch[:, :, 1],
                                op=MAX)
        wmv = wm.rearrange("p (h t w) -> p h t w", t=2, w=W // 2)
        po = pgps[b][:, mjs].rearrange("p (h w) -> p h w", w=W // 2)
        nc.vector.tensor_tensor(out=po, in0=wmv[:, :, 0, :],
                                in1=wmv[:, :, 1, :], op=MAX)

    def tr_step(b, c):
        # transpose pooled-g m-tile c into the g_aug^T layout (fp8 cast
        # happens in the copy; the O matmuls run fp8 DoubleRow)
        gt = pjut.tile([MT, CG], BF16, tag="pjut", name=f"gt{b}_{c}")
        nc.tensor.transpose(gt, pgps[b][0:CG, c * MT:(c + 1) * MT], ident)
        nc.vector.tensor_copy(out=gaTs[b][:, c, 0:32], in_=gt)

    ets = {}   # global slot -> et tile
    ots = {}   # (b, j) -> O_aug accumulator

    def slot_jg(K):
        # chunk-pair interleave: adjacent slots process the two chunks of
        # a pair at the same group, so paired O matmuls (and paired W_o
        # matmuls) reuse the PE's loaded weights
        b, k = K // (NCH * NG), K % (NCH * NG)
        p, q = k // 8, k % 8
        return b, 2 * p + q % 2, q // 2

    def st_exp(K):
        b, j, g = slot_jg(K)
        js = slice(j * NC, (j + 1) * NC)
        st = stp.tile([MT, GRP * NC], F32, tag="st", name=f"st{K}")
        for t in range(GRP):
            i = GRP * g + t
            nc.tensor.matmul(
                st[:, t * NC:(t + 1) * NC],
                pgps[b][32:40, i * MT:(i + 1) * MT],
                projs[b][32:40, js],
                start=True, stop=True)
        et = ep.tile([MT, GRP * NC], FP8 if USE_FP8_O else BF16,
                     tag="et", name=f"et{K}")
        nc.scalar.activation(out=et, in_=st, func=EXP)
        ets[K] = et

    def o_step(K):
        # one fp8 DoubleRow matmul contracts both m-tiles of the group:
        # lhsT [128, 2, GAW], rhs [128, 2, 512] -> out [GAW, 512]
        b, j, g = slot_jg(K)
        if g == 0:
            ots[(b, j)] = otp.tile([GAW if USE_FP8_O else 33, NC], F32,
                                   tag="ot", name=f"ot{b}_{j}")
        ot = ots[(b, j)]
        et = ets.pop(K)
        if USE_FP8_O:
            nc.tensor.matmul(ot, gaTs[b][:, GRP * g:GRP * (g + 1), :],
                             et.rearrange("p (two n) -> p two n", two=GRP),
                             start=(g == 0), stop=(g == NG - 1),
                             perf_mode=DBLROW)
        else:
            for t in range(GRP):
                i = GRP * g + t
                nc.tensor.matmul(ot, gaTs[b][:, i, :],
                                 et[:, t * NC:(t + 1) * NC],
                                 start=(i == 0), stop=(i == MTS - 1))

    def tail_norm(b, j):
        # o_norm = O[0:32] * (1/denom): recip (DVE) -> partition broadcast
        # (Pool) -> multiply (DVE)
        js = slice(j * NC, (j + 1) * NC)
        ot = ots[(b, j)]
        rs = sm.tile([1, NC], F32, tag="rs", name=f"rs{b}_{j}")
        nc.vector.reciprocal(out=rs, in_=ot[32:33, :])
        r32 = sm.tile([CG, NC], F32, tag="r32", name=f"r32{b}_{j}")
        nc.gpsimd.partition_broadcast(r32, rs)
        nc.vector.tensor_tensor(out=onorms[b][:, js], in0=ot[0:32, :],
                                in1=r32, op=MULT)

    def tail_out(b, j):
        # out = (gamma*W_o @ o + gamma*b_o) + x  (gamma folded into wot,
        # gamma*b_o applied as a per-partition scalar in the final add)
        js = slice(j * NC, (j + 1) * NC)
        del ots[(b, j)]
        ut = pjut.tile([C, NC], F32, tag="pjut", name=f"ut{b}_{j}")
        nc.tensor.matmul(ut, wot, onorms[b][:, js], start=True, stop=True)
        nc.vector.scalar_tensor_tensor(out=outbs[b][:, js], in0=ut,
                                       scalar=gbo, in1=xbfs[b][:, js],
                                       op0=ADD, op1=ADD)
        if j == NCH // 2 - 1:
            nc.sync.dma_start(out=out_d[b][:, 0:N // 2],
                              in_=outbs[b][:, 0:N // 2])
        elif b == BPC - 1 and j == NCH - 2:
            # keep the very last transfer small: it sits on the drain path
            nc.sync.dma_start(out=out_d[b][:, N // 2:N - NC],
                              in_=outbs[b][:, N // 2:N - NC])
        elif j == NCH - 1:
            if b == BPC - 1:
                nc.sync.dma_start(out=out_d[b][:, N - NC:N],
                                  in_=outbs[b][:, N - NC:N])
            else:
                nc.sync.dma_start(out=out_d[b][:, N // 2:N],
                                  in_=outbs[b][:, N // 2:N])

    # ---- emission schedule ----
    KPB = NCH * NG  # 32 slots per batch
    # batch 1 projection steps paced through batch 0's attention slots:
    # even slots only, so the PE extras never collide with the W_o
    # matmuls of the per-chunk tails (odd slots)
    b1_proj = {4 + 3 * c: c for c in range(NCH)}
    b1_tr = {6 + 3 * c: c for c in range(NCH)}

    proj_step(0, 0)
    proj_step(0, 1)
    for K in range(SLOTS):
        if K <= 3 and K >= 1:
            # batch-0 fill: remaining projection chunks ahead of their
            # first use; transposes one slot after their maxpools
            proj_step(0, 2 * K)
            proj_step(0, 2 * K + 1)
        st_exp(K)
        if 1 <= K <= 4:
            tr_step(0, 2 * K - 2)
            tr_step(0, 2 * K - 1)
        if K in b1_proj:
            proj_step(1, b1_proj[K])
        if K in b1_tr:
            tr_step(1, b1_tr[K])
        # pair tails first so the next pair's ot allocation (in the O
        # emission below) sequences after this pair's last reads
        for b in range(BPC):
            k = K - b * KPB
            if k >= 10 and (k - 10) % 8 == 0 and (k - 10) // 8 < NCH // 2:
                pp = (k - 10) // 8
                tail_norm(b, 2 * pp)
                tail_norm(b, 2 * pp + 1)
            if k >= 12 and (k - 12) % 8 == 0 and (k - 12) // 8 < NCH // 2:
                pp = (k - 12) // 8
                tail_out(b, 2 * pp)
                tail_out(b, 2 * pp + 1)
        if K >= 3 and (K - 3) % 2 == 0:
            o_step(K - 3)
            o_step(K - 2)
    # epilogue: flush the last two O groups, then drain the final chunk's
    # tail as two 256-col half-chains so DVE/Pool/PE/DMA pipeline
    o_step(SLOTS - 2)
    o_step(SLOTS - 1)
    tail_norm(BPC - 1, NCH - 2)
    tail_out(BPC - 1, NCH - 2)
    b, j = BPC - 1, NCH - 1
    ot = ots[(b, j)]
    HC = NC // 2
    rss, r32s = [], []
    for h in range(2):
        hs = slice(h * HC, (h + 1) * HC)
        rs = sm.tile([1, HC], F32, tag="rs", name=f"rse{h}")
        nc.vector.reciprocal(out=rs, in_=ot[32:33, hs])
        rss.append(rs)
    for h in range(2):
        r32 = sm.tile([CG, HC], F32, tag="r32", name=f"r32e{h}")
        nc.gpsimd.partition_broadcast(r32, rss[h])
        r32s.append(r32)
    for h in range(2):
        lo = j * NC + h * HC
        hs = slice(h * HC, (h + 1) * HC)
        gs = slice(lo, lo + HC)
        nc.vector.tensor_tensor(out=onorms[b][:, gs], in0=ot[0:32, hs],
                                in1=r32s[h], op=MULT)
        ut = pjut.tile([C, HC], F32, tag="pjut", name=f"ute{h}")
        nc.tensor.matmul(ut, wot, onorms[b][:, gs], start=True, stop=True)
        nc.vector.scalar_tensor_tensor(out=outbs[b][:, gs], in0=ut,
                                       scalar=gbo, in1=xbfs[b][:, gs],
                                       op0=ADD, op1=ADD)
        nc.sync.dma_start(out=out_d[b][:, gs], in_=outbs[b][:, gs])
    del ots[(b, j)]


_NC_CACHE = None


def _get_nc():
    global _NC_CACHE
    if _NC_CACHE is None:
        _NC_CACHE = build_bass()
    return _NC_CACHE


def prep_in_maps(inputs, W_theta, b_theta, W_phi, b_phi, W_g, b_g, W_o, b_o,
                 gamma, **_unused):
    inputs = np.asarray(inputs, np.float32)
    W_all = np.zeros((104, C), np.float32)
    W_all[32:32 + CT] = np.asarray(W_theta, np.float32)
    W_all[64:64 + CG] = np.asarray(W_g, np.float32)
    W_all[96:96 + CT] = np.asarray(W_phi, np.float32)
    g = np.float32(np.asarray(gamma, np.float32))

    const_bf = np.zeros((104, 168), np.float32)
    const_bf[0:C, 0:104] = W_all.T
    const_bf[0:CG, 104:168] = np.asarray(W_o, np.float32).T * g
    const_bf = np.ascontiguousarray(const_bf.astype(ml_dtypes.bfloat16))

    const_f32 = np.zeros((104, 2), np.float32)
    const_f32[32:32 + CT, 0] = np.asarray(b_theta, np.float32)
    const_f32[64:64 + CG, 0] = np.asarray(b_g, np.float32)
    const_f32[96:96 + CT, 0] = np.asarray(b_phi, np.float32)
    const_f32[0:C, 1] = np.asarray(b_o, np.float32) * g
    const_f32 = np.ascontiguousarray(const_f32)

    xbf = inputs.reshape(B, C, N).astype(ml_dtypes.bfloat16)
    in_maps = []
    for c in range(NCORES):
        in_maps.append({
            "xbf": np.ascontiguousarray(xbf[c * BPC:(c + 1) * BPC]),
            "const_bf": const_bf,
            "const_f32": const_f32,
        })
    return in_maps


def kernel(**inputs):
    in_maps = prep_in_maps(**inputs)
    nc = _get_nc()
    res = run_bass_kernel_spmd(nc, in_maps, core_ids=list(range(NCORES)))
    out = np.concatenate([res.results[c]["out"] for c in range(NCORES)],
                         axis=0)
    return out.reshape(B, C, H, W).astype(np.float32)


if __name__ == "__main__":
    rng = np.random.default_rng(0)
    ins = {
        "inputs": rng.standard_normal((B, C, H, W)).astype(np.float32),
        "W_theta": (rng.standard_normal((CT, C)) * 0.05).astype(np.float32),
        "b_theta": np.zeros(CT, np.float32),
        "W_phi": (rng.standard_normal((CT, C)) * 0.05).astype(np.float32),
        "b_phi": np.zeros(CT, np.float32),
        "W_g": (rng.standard_normal((CG, C)) * 0.05).astype(np.float32),
        "b_g": np.zeros(CG, np.float32),
        "W_o": (rng.standard_normal((C, CG)) * 0.05).astype(np.float32),
        "b_o": np.zeros(C, np.float32),
        "gamma": np.float32(0.5),
    }
    print(kernel(**ins).shape)
